# revision 1
# baseline (speedup 1.0000x reference)
"""Distributed GAT GNN kernel for 8 TRN2 NeuronCores (self-contained).

Algorithm (per core c, SPMD single program, per-core data via inputs):
  - T1[n] = [x@W1 | x@W1As1 | x@W1Ad1]  (bf16 table, replicated compute)
  - Layer-1 message passing for own dst nodes [c*6250, (c+1)*6250):
    dst-sorted edges grouped in 49 blocks of 128 dsts; per 128-edge chunk a
    dma_gather fetches [h|a_s] rows by src; attention softmax is computed
    without segment-max (scores are tiny); aggregation is a one-hot matmul
    accumulated in PSUM; denominators ride as extra matmul columns.
  - T2 local rows from relu(out1) (BN folded on host), AllGather -> T2 full.
  - Layer-2 message passing + per-graph pooling via one-hot matmuls,
    AllReduce of pooled sums, tiny MLP head. Output [500, 2] fp32.
"""
import sys

import numpy as np
from ml_dtypes import bfloat16

for _p in ("/opt/trn_rl_repo",):
    if _p not in sys.path:
        sys.path.append(_p)

import concourse.bass as bass
import concourse.tile as tile
from concourse import bacc, bass_utils, mybir

F32 = mybir.dt.float32
BF16 = mybir.dt.bfloat16
I16 = mybir.dt.int16
AF = mybir.ActivationFunctionType
OP = mybir.AluOpType

N = 50000
F_IN = 128
HID = 64
HEADS = 4
HC = HEADS * HID            # 256
OUT_DIM = 128
N_CLS = 2
NG = 500
SLOPE = 0.2
EPS = 1e-5
NCORES = 8
NLOC = N // NCORES          # 6250
NBLK = (NLOC + 127) // 128  # 49
LAST_VALID = NLOC - (NBLK - 1) * 128  # 106
SPLIT = 32768
TCOLS = 384                 # bf16 table row stride (768B)
UCOLS = 264                 # used columns [h(256)|a_s(4)|a_d(4)]
NPAD = 50176                # 98*512
NPAD_LOC = NBLK * 128       # 6272


def _bf(x):
    return np.ascontiguousarray(np.asarray(x, np.float32).astype(bfloat16))


def _f32(x):
    return np.ascontiguousarray(np.asarray(x, np.float32))


# ---------------------------------------------------------------- host prep
def preprocess_graph(edge_index, batch):
    src = np.asarray(edge_index[0], np.int64)
    dst = np.asarray(edge_index[1], np.int64)
    loop = np.arange(N, dtype=np.int64)
    src = np.concatenate([src, loop])
    dst = np.concatenate([dst, loop])

    core_of = dst // NLOC
    per_core = []
    for c in range(NCORES):
        m = core_of == c
        s, d = src[m], dst[m] - c * NLOC
        o = np.argsort(d, kind="stable")
        per_core.append((s[o], d[o]))

    lists = [[None] * NBLK for _ in range(NCORES)]
    GA = np.zeros(NBLK, np.int64)
    GB = np.zeros(NBLK, np.int64)
    for c in range(NCORES):
        s, d = per_core[c]
        blk = d // 128
        bnd = np.searchsorted(blk, np.arange(NBLK + 1))
        for b in range(NBLK):
            sb = s[bnd[b]:bnd[b + 1]]
            db = d[bnd[b]:bnd[b + 1]] - b * 128
            mA = sb < SPLIT
            lists[c][b] = ((sb[mA], db[mA]), (sb[~mA] - SPLIT, db[~mA]))
            GA[b] = max(GA[b], (len(sb[mA]) + 127) // 128)
            GB[b] = max(GB[b], (len(sb) - len(sb[mA]) + 127) // 128)
    blocks = [(int(GA[b]), int(GB[b])) for b in range(NBLK)]
    CH = int(GA.sum() + GB.sum())
    L = CH * 128

    idx16_l, dslot_l, oTt_l, bslot_l = [], [], [], []
    dvals = np.arange(128, dtype=np.float32)[:, None]
    batch = np.asarray(batch, np.int64)
    for c in range(NCORES):
        idx = np.zeros(L, np.int16)
        slo = np.full(L, -1.0, np.float32)
        off = 0
        for b in range(NBLK):
            for part in range(2):
                g = blocks[b][part]
                s, dsl = lists[c][b][part]
                n = len(s)
                idx[off:off + n] = s.astype(np.int16)
                slo[off:off + n] = dsl.astype(np.float32)
                off += g * 128
        assert off == L
        idx16_l.append(np.tile(idx.reshape(L // 16, 16).T, (8, 1)))       # [128, L/16]
        dslot_l.append(slo.reshape(CH, 128).T.astype(bfloat16))           # [128, CH]
        oTt_l.append((slo[None, :] == dvals).astype(bfloat16))            # [128, L]
        bs = np.full((128, NBLK), -1.0, np.float32)
        loc = batch[c * NLOC:(c + 1) * NLOC]
        for b in range(NBLK):
            seg = loc[b * 128:(b + 1) * 128]
            bs[:len(seg), b] = seg.astype(np.float32)
        bslot_l.append(bs)

    cnt = np.bincount(batch, minlength=NG).astype(np.float32)
    invcnt = 1.0 / np.clip(cnt, 1.0, None)
    return dict(blocks=blocks, CH=CH, L=L, idx16=idx16_l, dslot=dslot_l,
                oTt=oTt_l, bslot=bslot_l, invcnt=invcnt)


def fold_weights(inp):
    g = lambda k: np.asarray(inp[k], np.float32)
    W1, as1, ad1, b1 = g("W1"), g("att_src1"), g("att_dst1"), g("b1")
    W2, as2, ad2, b2 = g("W2"), g("att_src2"), g("att_dst2"), g("b2")
    g1, be1, rm1, rv1 = g("g1"), g("be1"), g("rm1"), g("rv1")
    g2, be2, rm2, rv2 = g("g2"), g("be2"), g("rm2"), g("rv2")
    lw1, lb1, lw2, lb2 = g("lw1"), g("lb1"), g("lw2"), g("lb2")

    def att_cols(W, a):
        return np.stack(
            [W[:, h * HID:(h + 1) * HID] @ a[h] for h in range(HEADS)], axis=1)

    Wcat1 = np.concatenate([W1, att_cols(W1, as1), att_cols(W1, ad1)], axis=1)
    s1 = g1 / np.sqrt(rv1 + EPS)
    t1 = be1 - rm1 * s1
    Wcat2u = np.concatenate([W2, att_cols(W2, as2), att_cols(W2, ad2)], axis=1)
    Wcat2 = s1[:, None] * Wcat2u
    rcat2 = t1 @ Wcat2u
    s2 = g2 / np.sqrt(rv2 + EPS)
    t2 = be2 - rm2 * s2
    return dict(
        wc1=_bf(Wcat1), wc2=_bf(Wcat2),
        rc2rep=_f32(np.tile(rcat2[None, :], (128, 1))),
        b1rep=_f32(np.tile(b1[None, :], (128, 1))),
        b2rep=_f32(np.tile(b2[None, :], (128, 1))),
        lw1=_bf(s2[:, None] * lw1), lb1=_f32((t2 @ lw1 + lb1)[:, None]),
        lw2=_bf(lw2), lb2=_f32(lb2[:, None]),
    )


# ------------------------------------------------------------- bass program
def build_program(blocks, CH, L):
    import os
    PHASE = int(os.environ.get("KPHASE", "9"))
    KSUB = int(os.environ.get("KSUB", "9"))
    KBLK = int(os.environ.get("KBLK", str(NBLK)))
    nc = bacc.Bacc("TRN2", num_devices=NCORES)

    ein = lambda name, shape, dt: nc.dram_tensor(name, shape, dt, kind="ExternalInput")
    xbT = ein("xbT", [128, NPAD], BF16)
    xbTo = ein("xbTo", [128, NPAD_LOC], BF16)
    wc1 = ein("wc1", [128, UCOLS], BF16)
    wc2 = ein("wc2", [256, UCOLS], BF16)
    rc2rep = ein("rc2rep", [128, UCOLS], F32)
    b1rep = ein("b1rep", [128, HC], F32)
    b2rep = ein("b2rep", [128, HC], F32)
    lw1 = ein("lw1", [256, OUT_DIM], BF16)
    lb1 = ein("lb1", [OUT_DIM, 1], F32)
    lw2 = ein("lw2", [OUT_DIM, N_CLS], BF16)
    lb2 = ein("lb2", [N_CLS, 1], F32)
    icntrep = ein("icntrep", [128, NG], F32)
    irep = ein("irep", [128, 128], BF16)
    i5rep = ein("i5rep", [128, NG], F32)
    idx16 = ein("idx16", [128, L // 16], I16)
    dslot = ein("dslot", [128, CH], BF16)
    oTt = ein("oTt", [128, L], BF16)
    bslot = ein("bslot", [128, NBLK], F32)
    out_t = nc.dram_tensor("out", [NG, N_CLS], F32, kind="ExternalOutput")

    T1 = nc.dram_tensor("T1", [NPAD, TCOLS], BF16)
    h2d = nc.dram_tensor("h2d", [NPAD_LOC, HC], BF16)
    T2l = nc.dram_tensor("T2l", [NLOC, TCOLS], BF16)
    T2 = nc.dram_tensor("T2", [N, TCOLS], BF16, addr_space="Shared")
    plcl = nc.dram_tensor("plcl", [HC, NG], F32)
    prdc = nc.dram_tensor("prdc", [HC, NG], F32, addr_space="Shared")

    qbase = np.cumsum([0] + [a + b for a, b in blocks]).tolist()

    from contextlib import ExitStack
    with tile.TileContext(nc) as tc, ExitStack() as es:
        cp = es.enter_context(tc.tile_pool(name="cp", bufs=1))
        wp = es.enter_context(tc.tile_pool(name="wp", bufs=3))
        gp = es.enter_context(tc.tile_pool(name="gp", bufs=2))
        pp = es.enter_context(tc.tile_pool(name="pp", bufs=1, space="PSUM"))
        pp2 = es.enter_context(tc.tile_pool(name="pp2", bufs=2, space="PSUM"))

        # ---- constants into SBUF
        def cload(ap, shape, dt, tag):
            t = cp.tile(shape, dt, tag=tag)
            nc.sync.dma_start(out=t[:], in_=ap)
            return t

        wc1_s = cload(wc1[:, :], [128, UCOLS], BF16, "wc1")
        wc2_s = cload(wc2[:, :].rearrange("(k p) c -> p k c", p=128), [128, 2, UCOLS], BF16, "wc2")
        rc2_s = cload(rc2rep[:, :], [128, UCOLS], F32, "rc2")
        b1_s = cload(b1rep[:, :], [128, HC], F32, "b1")
        b2_s = cload(b2rep[:, :], [128, HC], F32, "b2")
        lw1_s = cload(lw1[:, :].rearrange("(k p) c -> p k c", p=128), [128, 2, OUT_DIM], BF16, "lw1")
        lb1_s = cload(lb1[:, :], [OUT_DIM, 1], F32, "lb1")
        lw2_s = cload(lw2[:, :], [OUT_DIM, N_CLS], BF16, "lw2")
        lb2_s = cload(lb2[:, :], [N_CLS, 1], F32, "lb2")
        icnt_s = cload(icntrep[:, :], [128, NG], F32, "icnt")
        irep_s = cload(irep[:, :], [128, 128], BF16, "irep")
        i5_s = cload(i5rep[:, :], [128, NG], F32, "i5")
        idx_s = cload(idx16[:, :], [128, L // 16], I16, "idx")
        ds_s = cload(dslot[:, :], [128, CH], BF16, "ds")
        bs_s = cload(bslot[:, :], [128, NBLK], F32, "bs")
        xbTo_s = cload(xbTo[:, :], [128, NPAD_LOC], BF16, "xbTo")

        # ---- phase B: T1 = xb @ Wcat1 (all nodes, replicated)
        for j in range(NPAD // 512 if PHASE >= 1 else 0):
            xt = wp.tile([128, 512], BF16, tag="xt")
            nc.sync.dma_start(out=xt[:], in_=xbT[:, j * 512:(j + 1) * 512])
            for s in range(4):
                ps = pp2.tile([128, UCOLS], F32, tag="tb", space="PSUM")
                nc.tensor.matmul(out=ps[:], lhsT=xt[:, s * 128:(s + 1) * 128],
                                 rhs=wc1_s[:], start=True, stop=True)
                tb = wp.tile([128, UCOLS], BF16, tag="tb_sb")
                nc.vector.tensor_copy(out=tb[:], in_=ps[:])
                r0 = j * 512 + s * 128
                nc.sync.dma_start(out=T1[r0:r0 + 128, 0:UCOLS], in_=tb[:])

        # ---- phase C: a_d(layer1) for own nodes
        adsb1 = cp.tile([128, NBLK, HEADS], BF16, tag="adsb1")
        for b in range(NBLK if PHASE >= 2 else 0):
            ps = pp2.tile([128, HEADS], F32, tag="adp", space="PSUM")
            nc.tensor.matmul(out=ps[:], lhsT=xbTo_s[:, b * 128:(b + 1) * 128],
                             rhs=wc1_s[:, 260:264], start=True, stop=True)
            nc.scalar.activation(out=adsb1[:, b, :], in_=ps[:], func=AF.Copy)

        adsb2 = cp.tile([128, NBLK, HEADS], BF16, tag="adsb2")

        # ---- message-passing layer emitter
        def emit_layer(Ttab, adsb, brep_s, post):
            for b in range(KBLK):
                nA, nB = blocks[b]
                G = nA + nB
                q0 = qbase[b]
                e0 = q0 * 128
                gbuf = gp.tile([128, G, TCOLS], BF16, tag="gbuf")
                if nA:
                    nc.gpsimd.dma_gather(
                        out_ap=gbuf[:, 0:nA, :], in_ap=Ttab[0:SPLIT, 0:TCOLS],
                        idxs_ap=idx_s[:, e0 // 16:(e0 + nA * 128) // 16],
                        num_idxs=nA * 128, num_idxs_reg=nA * 128,
                        elem_size=TCOLS, elem_step=TCOLS, single_packet=False)
                if nB:
                    eB = e0 + nA * 128
                    nc.gpsimd.dma_gather(
                        out_ap=gbuf[:, nA:G, :], in_ap=Ttab[SPLIT:N, 0:TCOLS],
                        idxs_ap=idx_s[:, eB // 16:(eB + nB * 128) // 16],
                        num_idxs=nB * 128, num_idxs_reg=nB * 128,
                        elem_size=TCOLS, elem_step=TCOLS, single_packet=False)
                if KSUB < 2:
                    continue
                oT = gp.tile([128, G, 128], BF16, tag="oT")
                nc.sync.dma_start(out=oT[:], in_=oTt[:, q0 * 128:(q0 + G) * 128])

                # a_d expand per chunk: [128e, 4] = O^T_g^T.T @ a_d_block
                adp = pp2.tile([128, G, HEADS], F32, tag="adp", space="PSUM")
                for g in range(G):
                    nc.tensor.matmul(out=adp[:, g, :], lhsT=oT[:, g, :],
                                     rhs=adsb[:, b, :], start=True, stop=True)
                if KSUB < 3:
                    continue
                # scores -> w = exp(leaky(a_s + a_d))
                esb = wp.tile([128, G, HEADS], F32, tag="esb")
                nc.vector.tensor_tensor(out=esb[:], in0=adp[:],
                                        in1=gbuf[:, :, 256:260], op=OP.add)
                wsb = wp.tile([128, G, HEADS], F32, tag="wsb")
                nc.vector.scalar_tensor_tensor(out=wsb[:], in0=esb[:], scalar=SLOPE,
                                               in1=esb[:], op0=OP.mult, op1=OP.max)
                wex = wp.tile([128, G, HEADS], F32, tag="wex")
                nc.scalar.activation(out=wex[:], in_=wsb[:], func=AF.Exp)
                if KSUB < 4:
                    continue
                # w into table cols 256:260 (denominator columns), scale h by w
                nc.vector.tensor_copy(out=gbuf[:, :, 256:260], in_=wex[:])
                hview = gbuf[:, :, 0:256].rearrange("p g (h c) -> p g h c", h=HEADS)
                nc.vector.tensor_tensor(out=hview, in0=hview,
                                        in1=wex[:].broadcast_to([128, G, HEADS, HID]),
                                        op=OP.mult)
                if KSUB < 5:
                    continue
                # aggregation
                agg = pp2.tile([128, 260], F32, tag="agg", space="PSUM")
                for g in range(G):
                    Og = wp.tile([128, 128], BF16, tag="Og")
                    nc.vector.tensor_tensor(
                        out=Og[:], in0=ds_s[:, q0 + g:q0 + g + 1].broadcast_to([128, 128]),
                        in1=irep_s[:], op=OP.is_equal)
                    nc.tensor.matmul(out=agg[:], lhsT=Og[:], rhs=gbuf[:, g, 0:260],
                                     start=(g == 0), stop=(g == G - 1))
                if KSUB < 6:
                    continue
                # out = num/den + b, relu -> bf16
                den = wp.tile([128, HEADS], F32, tag="den")
                nc.vector.tensor_scalar_add(out=den[:], in0=agg[:, 256:260], scalar1=1e-30)
                rec = wp.tile([128, HEADS], F32, tag="rec")
                nc.vector.reciprocal(out=rec[:], in_=den[:])
                osb = wp.tile([128, HC], F32, tag="osb")
                nc.vector.tensor_tensor(
                    out=osb[:].rearrange("p (h c) -> p h c", h=HEADS),
                    in0=agg[:, 0:256].rearrange("p (h c) -> p h c", h=HEADS),
                    in1=rec[:].broadcast_to([128, HEADS, HID]),
                    op=OP.mult)
                nc.vector.tensor_tensor(out=osb[:], in0=osb[:], in1=brep_s[:], op=OP.add)
                h2 = wp.tile([128, HC], BF16, tag="h2")
                nc.scalar.activation(out=h2[:], in_=osb[:], func=AF.Relu)
                post(b, h2)

        # ---- layer 1
        def post1(b, h2):
            nc.sync.dma_start(out=h2d[b * 128:(b + 1) * 128, :], in_=h2[:])

        if PHASE >= 3:
            emit_layer(T1, adsb1, b1_s, post1)

        # ---- T2 local build: T2l = h2 @ Wcat2 + rcat2 (+ a_d2 capture)
        for j in range(13 if PHASE >= 4 else 0):
            nn = 512 if j < 12 else 128
            xa = wp.tile([128, 512], BF16, tag="xa")
            xb2 = wp.tile([128, 512], BF16, tag="xb2")
            nc.sync.dma_start_transpose(out=xa[:, 0:nn], in_=h2d[j * 512:j * 512 + nn, 0:128])
            nc.sync.dma_start_transpose(out=xb2[:, 0:nn], in_=h2d[j * 512:j * 512 + nn, 128:256])
            for s in range(nn // 128):
                ci = j * 4 + s
                ps = pp2.tile([128, UCOLS], F32, tag="tb", space="PSUM")
                nc.tensor.matmul(out=ps[:], lhsT=xa[:, s * 128:(s + 1) * 128],
                                 rhs=wc2_s[:, 0, :], start=True, stop=False)
                nc.tensor.matmul(out=ps[:], lhsT=xb2[:, s * 128:(s + 1) * 128],
                                 rhs=wc2_s[:, 1, :], start=False, stop=True)
                tb = wp.tile([128, UCOLS], BF16, tag="tb_sb")
                nc.vector.tensor_tensor(out=tb[:], in0=ps[:], in1=rc2_s[:], op=OP.add)
                nc.scalar.activation(out=adsb2[:, ci, :], in_=tb[:, 260:264], func=AF.Copy)
                r0 = ci * 128
                rows = min(128, NLOC - r0)
                if rows > 0:
                    nc.sync.dma_start(out=T2l[r0:r0 + rows, 0:UCOLS], in_=tb[0:rows, :])

        # ---- AllGather T2
        if PHASE >= 5:
            nc.gpsimd.collective_compute(
                "AllGather", OP.bypass, replica_groups=[list(range(NCORES))],
                ins=[T2l[:, :]], outs=[T2[:, :]])

        # ---- layer 2 + pooling
        plA = pp.tile([128, NG], F32, tag="plA", space="PSUM")
        plB = pp.tile([128, NG], F32, tag="plB", space="PSUM")

        def post2(b, h2):
            Bm = wp.tile([128, NG], BF16, tag="Bm")
            nc.vector.tensor_tensor(
                out=Bm[:], in0=bs_s[:, b:b + 1].broadcast_to([128, NG]),
                in1=i5_s[:], op=OP.is_equal)
            nc.tensor.matmul(out=plA[:], lhsT=h2[:, 0:128], rhs=Bm[:],
                             start=(b == 0), stop=(b == NBLK - 1))
            nc.tensor.matmul(out=plB[:], lhsT=h2[:, 128:256], rhs=Bm[:],
                             start=(b == 0), stop=(b == NBLK - 1))

        if PHASE >= 6:
            emit_layer(T2, adsb2, b2_s, post2)

        # ---- pooled AllReduce + MLP head
        plsb = wp.tile([128, 2, NG], F32, tag="plsb")
        if PHASE >= 6:
            nc.vector.tensor_copy(out=plsb[:, 0, :], in_=plA[:])
            nc.vector.tensor_copy(out=plsb[:, 1, :], in_=plB[:])
        else:
            nc.vector.memset(plsb[:], 0.0)
        nc.sync.dma_start(out=plcl[:, :].rearrange("(k p) g -> p k g", p=128), in_=plsb[:])
        if PHASE >= 7:
            nc.gpsimd.collective_compute(
                "AllReduce", OP.add, replica_groups=[list(range(NCORES))],
                ins=[plcl[:, :]], outs=[prdc[:, :]])
        prsb = wp.tile([128, 2, NG], F32, tag="prsb")
        nc.sync.dma_start(out=prsb[:], in_=prdc[:, :].rearrange("(k p) g -> p k g", p=128))
        pbn = wp.tile([128, 2, NG], BF16, tag="pbn")
        nc.vector.tensor_tensor(out=pbn[:, 0, :], in0=prsb[:, 0, :], in1=icnt_s[:], op=OP.mult)
        nc.vector.tensor_tensor(out=pbn[:, 1, :], in0=prsb[:, 1, :], in1=icnt_s[:], op=OP.mult)
        zp = pp2.tile([128, NG], F32, tag="adp", space="PSUM")
        nc.tensor.matmul(out=zp[:], lhsT=lw1_s[:, 0, :], rhs=pbn[:, 0, :], start=True, stop=False)
        nc.tensor.matmul(out=zp[:], lhsT=lw1_s[:, 1, :], rhs=pbn[:, 1, :], start=False, stop=True)
        zT = wp.tile([128, NG], BF16, tag="zT")
        nc.scalar.activation(out=zT[:], in_=zp[:], func=AF.Relu, bias=lb1_s[:])
        op_ = pp2.tile([N_CLS, NG], F32, tag="agg", space="PSUM")
        nc.tensor.matmul(out=op_[:], lhsT=lw2_s[:], rhs=zT[:], start=True, stop=True)
        ofin = wp.tile([N_CLS, NG], F32, tag="ofin")
        nc.scalar.activation(out=ofin[:], in_=op_[:], func=AF.Identity, bias=lb2_s[:])
        nc.sync.dma_start(out=out_t[:, :].rearrange("n c -> c n"), in_=ofin[:])

    nc.finalize()
    return nc


# ---------------------------------------------------------------- kernel()
def _prepare(inputs):
    inp = {k: np.asarray(v) for k, v in inputs.items()}
    prep = preprocess_graph(inp["edge_index"], inp["batch"])
    fw = fold_weights(inp)

    nc = build_program(prep["blocks"], prep["CH"], prep["L"])

    x = np.asarray(inp["x"], np.float32)
    xbT_full = np.zeros((128, NPAD), bfloat16)
    xbT_full[:, :N] = x.T.astype(bfloat16)
    common = dict(
        xbT=xbT_full,
        wc1=fw["wc1"], wc2=fw["wc2"], rc2rep=fw["rc2rep"],
        b1rep=fw["b1rep"], b2rep=fw["b2rep"],
        lw1=fw["lw1"], lb1=fw["lb1"], lw2=fw["lw2"], lb2=fw["lb2"],
        icntrep=_f32(np.tile(prep["invcnt"][None, :], (128, 1))),
        irep=_bf(np.tile(np.arange(128, dtype=np.float32)[None, :], (128, 1))),
        i5rep=_f32(np.tile(np.arange(NG, dtype=np.float32)[None, :], (128, 1))),
    )
    in_maps = []
    for c in range(NCORES):
        xo = np.zeros((128, NPAD_LOC), bfloat16)
        xo[:, :NLOC] = xbT_full[:, c * NLOC:(c + 1) * NLOC]
        in_maps.append(dict(
            common,
            xbTo=xo,
            idx16=np.ascontiguousarray(prep["idx16"][c]),
            dslot=np.ascontiguousarray(prep["dslot"][c]),
            oTt=np.ascontiguousarray(prep["oTt"][c]),
            bslot=np.ascontiguousarray(prep["bslot"][c]),
        ))
    return nc, in_maps


def kernel(**inputs):
    nc, in_maps = _prepare(inputs)
    res = bass_utils.run_bass_kernel_spmd(nc, in_maps, core_ids=list(range(NCORES)))
    return np.asarray(res.results[0]["out"], np.float32)


def profile_run(**inputs):
    """Run with NTFF profiling; returns (output, exec_time_ns)."""
    nc, in_maps = _prepare(inputs)
    res = bass_utils.run_bass_kernel_spmd(
        nc, in_maps, core_ids=list(range(NCORES)), trace=True)
    return np.asarray(res.results[0]["out"], np.float32), res.exec_time_ns


if __name__ == "__main__":
    rng = np.random.default_rng(0)
    ei = rng.integers(0, N, (2, 800000)).astype(np.int64)
    bt = np.sort(rng.integers(0, NG, N)).astype(np.int64)
    p = preprocess_graph(ei, bt)
    print("CH", p["CH"], "L", p["L"])



# revision 12
# speedup vs baseline: 1.4228x; 1.4228x over previous
"""Distributed GAT GNN kernel for 8 TRN2 NeuronCores (self-contained).

Algorithm (per core c, SPMD single program, per-core data via inputs):
  - T1[n] = [x@W1 | x@W1As1 | x@W1Ad1]  (bf16 table, replicated compute)
  - Layer-1 message passing for own dst nodes [c*6250, (c+1)*6250):
    dst-sorted edges grouped in 49 blocks of 128 dsts; per 128-edge chunk a
    dma_gather fetches [h|a_s] rows by src; attention softmax is computed
    without segment-max (scores are tiny); aggregation is a one-hot matmul
    accumulated in PSUM; denominators ride as extra matmul columns.
  - T2 local rows from relu(out1) (BN folded on host), AllGather -> T2 full.
  - Layer-2 message passing + per-graph pooling via one-hot matmuls,
    AllReduce of pooled sums, tiny MLP head. Output [500, 2] fp32.
"""
import sys

import numpy as np
from ml_dtypes import bfloat16

for _p in ("/opt/trn_rl_repo",):
    if _p not in sys.path:
        sys.path.append(_p)

import concourse.bass as bass
import concourse.tile as tile
from concourse import bacc, bass_utils, mybir

F32 = mybir.dt.float32
BF16 = mybir.dt.bfloat16
I16 = mybir.dt.int16
AF = mybir.ActivationFunctionType
OP = mybir.AluOpType

N = 50000
F_IN = 128
HID = 64
HEADS = 4
HC = HEADS * HID            # 256
OUT_DIM = 128
N_CLS = 2
NG = 500
SLOPE = 0.2
EPS = 1e-5
NCORES = 8
NLOC = N // NCORES          # 6250
NBLK = (NLOC + 127) // 128  # 49
LAST_VALID = NLOC - (NBLK - 1) * 128  # 106
SPLIT = 32768
TCOLS = 384                 # bf16 table row stride (768B)
UCOLS = 264                 # used columns [h(256)|a_s(4)|a_d(4)]
NPAD = 50176                # 98*512
NPAD_LOC = NBLK * 128       # 6272


def _bf(x):
    return np.ascontiguousarray(np.asarray(x, np.float32).astype(bfloat16))


def _f32(x):
    return np.ascontiguousarray(np.asarray(x, np.float32))


# ---------------------------------------------------------------- host prep
def preprocess_graph(edge_index, batch):
    src = np.asarray(edge_index[0], np.int64)
    dst = np.asarray(edge_index[1], np.int64)
    loop = np.arange(N, dtype=np.int64)
    src = np.concatenate([src, loop])
    dst = np.concatenate([dst, loop])

    core_of = dst // NLOC
    per_core = []
    for c in range(NCORES):
        m = core_of == c
        s, d = src[m], dst[m] - c * NLOC
        o = np.argsort(d, kind="stable")
        per_core.append((s[o], d[o]))

    lists = [[None] * NBLK for _ in range(NCORES)]
    GA = np.zeros(NBLK, np.int64)
    GB = np.zeros(NBLK, np.int64)
    for c in range(NCORES):
        s, d = per_core[c]
        blk = d // 128
        bnd = np.searchsorted(blk, np.arange(NBLK + 1))
        for b in range(NBLK):
            sb = s[bnd[b]:bnd[b + 1]]
            db = d[bnd[b]:bnd[b + 1]] - b * 128
            mA = sb < SPLIT
            lists[c][b] = ((sb[mA], db[mA]), (sb[~mA] - SPLIT, db[~mA]))
            GA[b] = max(GA[b], (len(sb[mA]) + 127) // 128)
            GB[b] = max(GB[b], (len(sb) - len(sb[mA]) + 127) // 128)
    blocks = [(int(GA[b]), int(GB[b])) for b in range(NBLK)]
    CH = int(GA.sum() + GB.sum())
    L = CH * 128

    idx16_l, dslot_l, oTt_l, bslot_l = [], [], [], []
    dvals = np.arange(128, dtype=np.float32)[:, None]
    batch = np.asarray(batch, np.int64)
    for c in range(NCORES):
        idx = np.zeros(L, np.int16)
        slo = np.full(L, -1.0, np.float32)
        off = 0
        for b in range(NBLK):
            for part in range(2):
                g = blocks[b][part]
                s, dsl = lists[c][b][part]
                n = len(s)
                idx[off:off + n] = s.astype(np.int16)
                slo[off:off + n] = dsl.astype(np.float32)
                off += g * 128
        assert off == L
        idx16_l.append(np.tile(idx.reshape(L // 16, 16).T, (8, 1)))       # [128, L/16]
        dslot_l.append(slo.reshape(CH, 128).T.astype(bfloat16))           # [128, CH]
        oTt_l.append((slo[None, :] == dvals).astype(bfloat16))            # [128, L]
        bs = np.full((128, NBLK), -1.0, np.float32)
        loc = batch[c * NLOC:(c + 1) * NLOC]
        for b in range(NBLK):
            seg = loc[b * 128:(b + 1) * 128]
            bs[:len(seg), b] = seg.astype(np.float32)
        bslot_l.append(bs)

    cnt = np.bincount(batch, minlength=NG).astype(np.float32)
    invcnt = 1.0 / np.clip(cnt, 1.0, None)
    return dict(blocks=blocks, CH=CH, L=L, idx16=idx16_l, dslot=dslot_l,
                oTt=oTt_l, bslot=bslot_l, invcnt=invcnt)


def fold_weights(inp):
    g = lambda k: np.asarray(inp[k], np.float32)
    W1, as1, ad1, b1 = g("W1"), g("att_src1"), g("att_dst1"), g("b1")
    W2, as2, ad2, b2 = g("W2"), g("att_src2"), g("att_dst2"), g("b2")
    g1, be1, rm1, rv1 = g("g1"), g("be1"), g("rm1"), g("rv1")
    g2, be2, rm2, rv2 = g("g2"), g("be2"), g("rm2"), g("rv2")
    lw1, lb1, lw2, lb2 = g("lw1"), g("lb1"), g("lw2"), g("lb2")

    def att_cols(W, a):
        return np.stack(
            [W[:, h * HID:(h + 1) * HID] @ a[h] for h in range(HEADS)], axis=1)

    Wcat1 = np.concatenate([W1, att_cols(W1, as1), att_cols(W1, ad1)], axis=1)
    s1 = g1 / np.sqrt(rv1 + EPS)
    t1 = be1 - rm1 * s1
    Wcat2u = np.concatenate([W2, att_cols(W2, as2), att_cols(W2, ad2)], axis=1)
    Wcat2 = s1[:, None] * Wcat2u
    rcat2 = t1 @ Wcat2u
    s2 = g2 / np.sqrt(rv2 + EPS)
    t2 = be2 - rm2 * s2
    return dict(
        wc1=_bf(Wcat1), wc2=_bf(Wcat2),
        rc2rep=_f32(np.tile(rcat2[None, :], (128, 1))),
        b1rep=_f32(np.tile(b1[None, :], (128, 1))),
        b2rep=_f32(np.tile(b2[None, :], (128, 1))),
        lw1=_bf(s2[:, None] * lw1), lb1=_f32((t2 @ lw1 + lb1)[:, None]),
        lw2=_bf(lw2), lb2=_f32(lb2[:, None]),
    )


# ------------------------------------------------------------- bass program
def build_program(blocks, CH, L, GMAX):
    import os
    PHASE = int(os.environ.get("KPHASE", "9"))
    KSUB = int(os.environ.get("KSUB", "9"))
    KBLK = int(os.environ.get("KBLK", str(NBLK)))
    nc = bacc.Bacc("TRN2", num_devices=NCORES)

    ein = lambda name, shape, dt: nc.dram_tensor(name, shape, dt, kind="ExternalInput")
    xbT = ein("xbT", [128, NPAD], BF16)
    xbTo = ein("xbTo", [128, NPAD_LOC], BF16)
    wc1 = ein("wc1", [128, UCOLS], BF16)
    wc2 = ein("wc2", [256, UCOLS], BF16)
    rc2rep = ein("rc2rep", [128, UCOLS], F32)
    b1rep = ein("b1rep", [128, HC], F32)
    b2rep = ein("b2rep", [128, HC], F32)
    lw1 = ein("lw1", [256, OUT_DIM], BF16)
    lb1 = ein("lb1", [OUT_DIM, 1], F32)
    lw2 = ein("lw2", [OUT_DIM, N_CLS], BF16)
    lb2 = ein("lb2", [N_CLS, 1], F32)
    icntrep = ein("icntrep", [128, NG], F32)
    irep = ein("irep", [128, 128], BF16)
    irepb = ein("irepb", [128, GMAX * 128], BF16)
    identb = ein("identb", [128, 128], BF16)
    i5rep = ein("i5rep", [128, NG], F32)
    idx16 = ein("idx16", [128, L // 16], I16)
    dslot = ein("dslot", [128, CH], BF16)
    oTt = ein("oTt", [128, L], BF16)
    bslot = ein("bslot", [128, NBLK], F32)
    out_t = nc.dram_tensor("out", [NG, N_CLS], F32, kind="ExternalOutput")

    T1 = nc.dram_tensor("T1", [NPAD, TCOLS], BF16)
    T2l = nc.dram_tensor("T2l", [NLOC, TCOLS], BF16)
    T2 = nc.dram_tensor("T2", [N, TCOLS], BF16, addr_space="Shared")
    plcl = nc.dram_tensor("plcl", [HC, NG], F32)
    prdc = nc.dram_tensor("prdc", [HC, NG], F32, addr_space="Shared")

    qbase = np.cumsum([0] + [a + b for a, b in blocks]).tolist()

    from contextlib import ExitStack
    with tile.TileContext(nc) as tc, ExitStack() as es:
        cp = es.enter_context(tc.tile_pool(name="cp", bufs=1))
        wp = es.enter_context(tc.tile_pool(name="wp", bufs=3))
        gp = es.enter_context(tc.tile_pool(name="gp", bufs=3))
        pp = es.enter_context(tc.tile_pool(name="pp", bufs=1, space="PSUM"))
        pp2 = es.enter_context(tc.tile_pool(name="pp2", bufs=2, space="PSUM"))

        # ---- constants into SBUF
        def cload(ap, shape, dt, tag):
            t = cp.tile(shape, dt, tag=tag)
            nc.sync.dma_start(out=t[:], in_=ap)
            return t

        wc1_s = cload(wc1[:, :], [128, UCOLS], BF16, "wc1")
        wc2_s = cload(wc2[:, :].rearrange("(k p) c -> p k c", p=128), [128, 2, UCOLS], BF16, "wc2")
        rc2_s = cload(rc2rep[:, :], [128, UCOLS], F32, "rc2")
        b1_s = cload(b1rep[:, :], [128, HC], F32, "b1")
        b2_s = cload(b2rep[:, :], [128, HC], F32, "b2")
        lw1_s = cload(lw1[:, :].rearrange("(k p) c -> p k c", p=128), [128, 2, OUT_DIM], BF16, "lw1")
        lb1_s = cload(lb1[:, :], [OUT_DIM, 1], F32, "lb1")
        lw2_s = cload(lw2[:, :], [OUT_DIM, N_CLS], BF16, "lw2")
        lb2_s = cload(lb2[:, :], [N_CLS, 1], F32, "lb2")
        icnt_s = cload(icntrep[:, :], [128, NG], F32, "icnt")
        irep_s = cload(irep[:, :], [128, 128], BF16, "irep")
        irepb_s = cload(irepb[:, :], [128, GMAX * 128], BF16, "irepb")
        ident_s = cload(identb[:, :], [128, 128], BF16, "ident")
        i5_s = cload(i5rep[:, :], [128, NG], F32, "i5")
        idx_s = cload(idx16[:, :], [128, L // 16], I16, "idx")
        ds_s = cload(dslot[:, :], [128, CH], BF16, "ds")
        bs_s = cload(bslot[:, :], [128, NBLK], F32, "bs")
        xbTo_s = cload(xbTo[:, :], [128, NPAD_LOC], BF16, "xbTo")

        # ---- phase B: T1 = xb @ Wcat1 (all nodes, replicated)
        for j in range(NPAD // 512 if PHASE >= 1 else 0):
            eng = nc.sync if j % 2 == 0 else nc.scalar
            xt = wp.tile([128, 512], BF16, tag="xt")
            eng.dma_start(out=xt[:], in_=xbT[:, j * 512:(j + 1) * 512])
            tb4 = wp.tile([128, 4, UCOLS], BF16, tag="tb4")
            for s in range(4):
                ps = pp2.tile([128, UCOLS], F32, tag="tb", space="PSUM")
                nc.tensor.matmul(out=ps[:], lhsT=xt[:, s * 128:(s + 1) * 128],
                                 rhs=wc1_s[:], start=True, stop=True)
                nc.vector.tensor_copy(out=tb4[:, s, :], in_=ps[:])
            eng2 = nc.scalar if j % 2 == 0 else nc.sync
            eng2.dma_start(
                out=T1[j * 512:(j + 1) * 512, 0:UCOLS].rearrange(
                    "(s p) c -> p s c", p=128),
                in_=tb4[:])

        # ---- phase C: a_d(layer1) for own nodes
        adsb1 = cp.tile([128, NBLK, HEADS], BF16, tag="adsb1")
        for b in range(NBLK if PHASE >= 2 else 0):
            ps = pp2.tile([128, HEADS], F32, tag="adp", space="PSUM")
            nc.tensor.matmul(out=ps[:], lhsT=xbTo_s[:, b * 128:(b + 1) * 128],
                             rhs=wc1_s[:, 260:264], start=True, stop=True)
            nc.scalar.activation(out=adsb1[:, b, :], in_=ps[:], func=AF.Copy)

        adsb2 = cp.tile([128, NBLK, HEADS], BF16, tag="adsb2")

        # ---- message-passing layer emitter
        def emit_layer(Ttab, adsb, brep_s, post):
            for b in range(KBLK):
                nA, nB = blocks[b]
                G = nA + nB
                q0 = qbase[b]
                e0 = q0 * 128
                gbuf = gp.tile([128, G, TCOLS], BF16, tag="gbuf")
                if nA:
                    nc.gpsimd.dma_gather(
                        out_ap=gbuf[:, 0:nA, :], in_ap=Ttab[0:SPLIT, 0:TCOLS],
                        idxs_ap=idx_s[:, e0 // 16:(e0 + nA * 128) // 16],
                        num_idxs=nA * 128, num_idxs_reg=nA * 128,
                        elem_size=TCOLS, elem_step=TCOLS, single_packet=False)
                if nB:
                    eB = e0 + nA * 128
                    nc.gpsimd.dma_gather(
                        out_ap=gbuf[:, nA:G, :], in_ap=Ttab[SPLIT:N, 0:TCOLS],
                        idxs_ap=idx_s[:, eB // 16:(eB + nB * 128) // 16],
                        num_idxs=nB * 128, num_idxs_reg=nB * 128,
                        elem_size=TCOLS, elem_step=TCOLS, single_packet=False)
                if KSUB < 2:
                    continue
                oT = gp.tile([128, G, 128], BF16, tag="oT")
                nc.sync.dma_start(out=oT[:], in_=oTt[:, q0 * 128:(q0 + G) * 128])

                # a_d expand per chunk: [128e, 4] = O^T_g^T.T @ a_d_block
                adp = pp2.tile([128, G, HEADS], F32, tag="adp", space="PSUM")
                for g in range(G):
                    nc.tensor.matmul(out=adp[:, g, :], lhsT=oT[:, g, :],
                                     rhs=adsb[:, b, :], start=True, stop=True)
                if KSUB < 3:
                    continue
                # scores -> w = exp(leaky(a_s + a_d))
                esb = wp.tile([128, G, HEADS], F32, tag="esb")
                nc.vector.tensor_tensor(out=esb[:], in0=adp[:],
                                        in1=gbuf[:, :, 256:260], op=OP.add)
                wsb = wp.tile([128, G, HEADS], F32, tag="wsb")
                nc.vector.scalar_tensor_tensor(out=wsb[:], in0=esb[:], scalar=SLOPE,
                                               in1=esb[:], op0=OP.mult, op1=OP.max)
                wex = wp.tile([128, G, HEADS], F32, tag="wex")
                nc.scalar.activation(out=wex[:], in_=wsb[:], func=AF.Exp)
                if KSUB < 4:
                    continue
                # w into table cols 256:260 (denominator columns), scale h by w
                nc.vector.tensor_copy(out=gbuf[:, :, 256:260], in_=wex[:])
                hview = gbuf[:, :, 0:256].rearrange("p g (h c) -> p g h c", h=HEADS)
                nc.vector.tensor_tensor(out=hview, in0=hview,
                                        in1=wex[:].broadcast_to([128, G, HEADS, HID]),
                                        op=OP.mult)
                if KSUB < 5:
                    continue
                # aggregation (edge->slot one-hots built in ONE vector op)
                og = gp.tile([128, G, 128], BF16, tag="og")
                nc.vector.tensor_tensor(
                    out=og[:], in0=ds_s[:, q0:q0 + G].broadcast_to([128, G, 128]),
                    in1=irepb_s[:, 0:G * 128].rearrange("p (g j) -> p g j", j=128),
                    op=OP.is_equal)
                agg = pp2.tile([128, 260], F32, tag="agg", space="PSUM")
                for g in range(G):
                    nc.tensor.matmul(out=agg[:], lhsT=og[:, g, :], rhs=gbuf[:, g, 0:260],
                                     start=(g == 0), stop=(g == G - 1))
                if KSUB < 6:
                    continue
                # out = num/den + b, relu -> bf16
                den = wp.tile([128, HEADS], F32, tag="den")
                nc.vector.tensor_scalar_add(out=den[:], in0=agg[:, 256:260], scalar1=1e-30)
                rec = wp.tile([128, HEADS], F32, tag="rec")
                nc.vector.reciprocal(out=rec[:], in_=den[:])
                osb = wp.tile([128, HC], F32, tag="osb")
                nc.vector.tensor_tensor(
                    out=osb[:].rearrange("p (h c) -> p h c", h=HEADS),
                    in0=agg[:, 0:256].rearrange("p (h c) -> p h c", h=HEADS),
                    in1=rec[:].broadcast_to([128, HEADS, HID]),
                    op=OP.mult)
                nc.vector.tensor_tensor(out=osb[:], in0=osb[:], in1=brep_s[:], op=OP.add)
                h2 = wp.tile([128, HC], BF16, tag="h2")
                nc.scalar.activation(out=h2[:], in_=osb[:], func=AF.Relu)
                post(b, h2)

        # ---- layer 1: fold the T2-row build into each block's tail so the
        # whole T2l table is ready ~when the last L1 gather lands.
        def post1(b, h2):
            pt = pp2.tile([128, 2, 128], F32, tag="tb", space="PSUM")
            nc.tensor.matmul(out=pt[:, 0, :], lhsT=h2[:, 0:128], rhs=ident_s[:],
                             start=True, stop=True)
            nc.tensor.matmul(out=pt[:, 1, :], lhsT=h2[:, 128:256], rhs=ident_s[:],
                             start=True, stop=True)
            h2T = wp.tile([128, 2, 128], BF16, tag="h2T")
            nc.vector.tensor_copy(out=h2T[:], in_=pt[:])
            ps = pp2.tile([128, UCOLS], F32, tag="tb", space="PSUM")
            nc.tensor.matmul(out=ps[:], lhsT=h2T[:, 0, :], rhs=wc2_s[:, 0, :],
                             start=True, stop=False)
            nc.tensor.matmul(out=ps[:], lhsT=h2T[:, 1, :], rhs=wc2_s[:, 1, :],
                             start=False, stop=True)
            tb = wp.tile([128, UCOLS], BF16, tag="tb_sb")
            nc.vector.tensor_tensor(out=tb[:], in0=ps[:], in1=rc2_s[:], op=OP.add)
            nc.scalar.activation(out=adsb2[:, b, :], in_=tb[:, 260:264], func=AF.Copy)
            r0 = b * 128
            rows = min(128, NLOC - r0)
            eng = nc.sync if b % 2 == 0 else nc.scalar
            eng.dma_start(out=T2l[r0:r0 + rows, 0:UCOLS], in_=tb[0:rows, :])

        if PHASE >= 3:
            emit_layer(T1, adsb1, b1_s, post1)

        # ---- AllGather T2
        if PHASE >= 5:
            nc.gpsimd.collective_compute(
                "AllGather", OP.bypass, replica_groups=[list(range(NCORES))],
                ins=[T2l[:, :]], outs=[T2[:, :]])

        # ---- layer 2 + pooling
        plA = pp.tile([128, NG], F32, tag="plA", space="PSUM")
        plB = pp.tile([128, NG], F32, tag="plB", space="PSUM")

        def post2(b, h2):
            Bm = wp.tile([128, NG], BF16, tag="Bm")
            nc.vector.tensor_tensor(
                out=Bm[:], in0=bs_s[:, b:b + 1].broadcast_to([128, NG]),
                in1=i5_s[:], op=OP.is_equal)
            nc.tensor.matmul(out=plA[:], lhsT=h2[:, 0:128], rhs=Bm[:],
                             start=(b == 0), stop=(b == NBLK - 1))
            nc.tensor.matmul(out=plB[:], lhsT=h2[:, 128:256], rhs=Bm[:],
                             start=(b == 0), stop=(b == NBLK - 1))

        if PHASE >= 6:
            emit_layer(T2, adsb2, b2_s, post2)

        # ---- pooled AllReduce + MLP head
        plsb = wp.tile([128, 2, NG], F32, tag="plsb")
        if PHASE >= 6:
            nc.vector.tensor_copy(out=plsb[:, 0, :], in_=plA[:])
            nc.vector.tensor_copy(out=plsb[:, 1, :], in_=plB[:])
        else:
            nc.vector.memset(plsb[:], 0.0)
        nc.sync.dma_start(out=plcl[:, :].rearrange("(k p) g -> p k g", p=128), in_=plsb[:])
        if PHASE >= 7:
            nc.gpsimd.collective_compute(
                "AllReduce", OP.add, replica_groups=[list(range(NCORES))],
                ins=[plcl[:, :]], outs=[prdc[:, :]])
        prsb = wp.tile([128, 2, NG], F32, tag="prsb")
        nc.sync.dma_start(out=prsb[:], in_=prdc[:, :].rearrange("(k p) g -> p k g", p=128))
        pbn = wp.tile([128, 2, NG], BF16, tag="pbn")
        nc.vector.tensor_tensor(out=pbn[:, 0, :], in0=prsb[:, 0, :], in1=icnt_s[:], op=OP.mult)
        nc.vector.tensor_tensor(out=pbn[:, 1, :], in0=prsb[:, 1, :], in1=icnt_s[:], op=OP.mult)
        zp = pp2.tile([128, NG], F32, tag="adp", space="PSUM")
        nc.tensor.matmul(out=zp[:], lhsT=lw1_s[:, 0, :], rhs=pbn[:, 0, :], start=True, stop=False)
        nc.tensor.matmul(out=zp[:], lhsT=lw1_s[:, 1, :], rhs=pbn[:, 1, :], start=False, stop=True)
        zT = wp.tile([128, NG], BF16, tag="zT")
        nc.scalar.activation(out=zT[:], in_=zp[:], func=AF.Relu, bias=lb1_s[:])
        op_ = pp2.tile([N_CLS, NG], F32, tag="agg", space="PSUM")
        nc.tensor.matmul(out=op_[:], lhsT=lw2_s[:], rhs=zT[:], start=True, stop=True)
        ofin = wp.tile([N_CLS, NG], F32, tag="ofin")
        nc.scalar.activation(out=ofin[:], in_=op_[:], func=AF.Identity, bias=lb2_s[:])
        nc.sync.dma_start(out=out_t[:, :].rearrange("n c -> c n"), in_=ofin[:])

    nc.finalize()
    return nc


# ---------------------------------------------------------------- kernel()
def _prepare(inputs):
    inp = {k: np.asarray(v) for k, v in inputs.items()}
    prep = preprocess_graph(inp["edge_index"], inp["batch"])
    fw = fold_weights(inp)
    GMAX = max(a + b for a, b in prep["blocks"])

    nc = build_program(prep["blocks"], prep["CH"], prep["L"], GMAX)

    x = np.asarray(inp["x"], np.float32)
    xbT_full = np.zeros((128, NPAD), bfloat16)
    xbT_full[:, :N] = x.T.astype(bfloat16)
    common = dict(
        xbT=xbT_full,
        wc1=fw["wc1"], wc2=fw["wc2"], rc2rep=fw["rc2rep"],
        b1rep=fw["b1rep"], b2rep=fw["b2rep"],
        lw1=fw["lw1"], lb1=fw["lb1"], lw2=fw["lw2"], lb2=fw["lb2"],
        icntrep=_f32(np.tile(prep["invcnt"][None, :], (128, 1))),
        irep=_bf(np.tile(np.arange(128, dtype=np.float32)[None, :], (128, 1))),
        irepb=_bf(np.tile(np.arange(128, dtype=np.float32)[None, :], (128, GMAX))),
        identb=_bf(np.eye(128, dtype=np.float32)),
        i5rep=_f32(np.tile(np.arange(NG, dtype=np.float32)[None, :], (128, 1))),
    )
    in_maps = []
    for c in range(NCORES):
        xo = np.zeros((128, NPAD_LOC), bfloat16)
        xo[:, :NLOC] = xbT_full[:, c * NLOC:(c + 1) * NLOC]
        in_maps.append(dict(
            common,
            xbTo=xo,
            idx16=np.ascontiguousarray(prep["idx16"][c]),
            dslot=np.ascontiguousarray(prep["dslot"][c]),
            oTt=np.ascontiguousarray(prep["oTt"][c]),
            bslot=np.ascontiguousarray(prep["bslot"][c]),
        ))
    return nc, in_maps


def kernel(**inputs):
    nc, in_maps = _prepare(inputs)
    res = bass_utils.run_bass_kernel_spmd(nc, in_maps, core_ids=list(range(NCORES)))
    return np.asarray(res.results[0]["out"], np.float32)


def profile_run(**inputs):
    """Run with NTFF profiling; returns (output, exec_time_ns)."""
    nc, in_maps = _prepare(inputs)
    res = bass_utils.run_bass_kernel_spmd(
        nc, in_maps, core_ids=list(range(NCORES)), trace=True)
    return np.asarray(res.results[0]["out"], np.float32), res.exec_time_ns


if __name__ == "__main__":
    rng = np.random.default_rng(0)
    ei = rng.integers(0, N, (2, 800000)).astype(np.int64)
    bt = np.sort(rng.integers(0, NG, N)).astype(np.int64)
    p = preprocess_graph(ei, bt)
    print("CH", p["CH"], "L", p["L"])

